# revision 1
# baseline (speedup 1.0000x reference)
"""Multi-head attention (B=2, S=2048, D=1024, H=16) on 8 TRN2 NeuronCores.

Sharding: data-parallel over batch (2) x tensor-parallel over head groups
(4 groups of 4 heads).  Core c = (b = c // 4, g = c % 4).  Each core:
  q/k/v = x[b] @ W{q,k,v}[:, 256g:256g+256] + b{q,k,v}[...]   (1/sqrt(dh)
  folded into Wq/bq on host), per-head softmax(q k^T) v, then a partial
  out-projection y_c = attn_out @ Wo[256g:256g+256, :].  Host sums the 4
  partials per batch and adds bo.

Device layouts (per core):
  xT   [1024, 2048]  (x[b] transposed on host)
  qT/kT: [256, 2048] as 2 sbuf tiles [128, 2048] (head h -> partitions
         64*(h%2).. of tile h//2)
  v_ext: 16 tiles [128, 260]; head h at cols 65h..65h+63, ones at 65h+64
         (ones column makes P @ V_ext also emit softmax denominators)
  scores^T per (head, ktile): [128, 2048] computed per q-half [128, 1024]
  attn_out^T: 2 tiles [128, 2048] (same head layout as qT)
  yT   [1024, 2048] partial output (ExternalOutput)
"""

import os
import sys
import types
from contextlib import ExitStack

import numpy as np

D = 1024
S = 2048
C = 256          # head cols per core (4 heads x 64)
DH = 64
NH = 4           # heads per core
QH = 1024        # q-half size
MM_DT_NAME = "float32r"   # matmul compute dtype (float32r = full-rate fp32)

_CACHE = {}


def _install_ntff_shim():
    try:
        import antenv.axon_hooks  # noqa: F401
        return
    except ImportError:
        pass
    try:
        from trn_agent_boot.trn_boot import _ntff_profile_via_ctypes
        hook = _ntff_profile_via_ctypes('/opt/axon/libaxon_pjrt.so')
    except Exception:
        hook = None
    mod = types.ModuleType('antenv.axon_hooks')
    mod.get_axon_ntff_profile_hook = lambda: hook
    mod.set_axon_ntff_profile_hook = lambda h: None
    sys.modules['antenv.axon_hooks'] = mod


def build_nc(seq=S):
    import concourse.bacc as bacc
    import concourse.mybir as mybir
    import concourse.tile as tile
    from concourse.bass import ts, ds

    F32 = mybir.dt.float32
    MM = getattr(mybir.dt, MM_DT_NAME)
    ACT = mybir.ActivationFunctionType

    nqb = seq // 512          # 512-wide column blocks of seq
    nst = seq // 128          # 128-row tiles of seq
    qh_w = min(QH, seq)       # q-half width
    nqh = seq // qh_w         # number of q halves
    qh_b = qh_w // 512        # 512-blocks per q half

    nc = bacc.Bacc("TRN2", target_bir_lowering=False, debug=False)
    xT = nc.dram_tensor("xT", [D, seq], MM, kind="ExternalInput")
    wq = nc.dram_tensor("wq", [D, C], MM, kind="ExternalInput")
    wk = nc.dram_tensor("wk", [D, C], MM, kind="ExternalInput")
    wv = nc.dram_tensor("wv", [D, C], MM, kind="ExternalInput")
    wo = nc.dram_tensor("wo", [C, D], MM, kind="ExternalInput")
    bqk = nc.dram_tensor("bqk", [128, 4], F32, kind="ExternalInput")  # [bq0 bq1 bk0 bk1]
    bv = nc.dram_tensor("bv", [1, C], F32, kind="ExternalInput")
    yT = nc.dram_tensor("yT", [D, seq], F32, kind="ExternalOutput")

    with tile.TileContext(nc) as tc, ExitStack() as ctx:
        consts = ctx.enter_context(tc.tile_pool(name="consts", bufs=1))
        sbw = ctx.enter_context(tc.tile_pool(name="weights", bufs=1))
        sbx = ctx.enter_context(tc.tile_pool(name="xT", bufs=1))
        sbqkv = ctx.enter_context(tc.tile_pool(name="qkv", bufs=1))
        sbpt = ctx.enter_context(tc.tile_pool(name="pt", bufs=3))
        sbnrm = ctx.enter_context(tc.tile_pool(name="nrm", bufs=2))
        sby = ctx.enter_context(tc.tile_pool(name="ysb", bufs=4))

        # ---- constants ----
        bqk_sb = consts.tile([128, 4], F32, tag="bqk", name="bqk_sb")
        nc.sync.dma_start(bqk_sb[:], bqk[:, :])
        bv_row = consts.tile([1, C], F32, tag="bvrow", name="bv_row")
        nc.sync.dma_start(bv_row[:], bv[:, :])
        bvb = consts.tile([128, C], F32, tag="bvb", name="bvb")
        nc.gpsimd.partition_broadcast(bvb[:], bv_row[:])
        ones4 = consts.tile([128, NH], F32, tag="ones4", name="ones4")
        nc.vector.memset(ones4[:], 1.0)

        # ---- loads: interleave weights with xT column blocks so the first
        #      projection group (wq + xt nb0) lands in ~10us ----
        xt_sb = [sbx.tile([128, seq], MM, tag=f"xt{i}", name=f"xt{i}")
                 for i in range(8)]
        w_sb = {}
        for name, dram in (("q", wq), ("k", wk), ("v", wv)):
            tiles = []
            for i in range(8):
                t = sbw.tile([128, C], MM, tag=f"w{name}{i}", name=f"w{name}{i}")
                nc.sync.dma_start(t[:], dram[ts(i, 128), :])
                tiles.append(t)
            w_sb[name] = tiles
            nbs = {"q": list(range(nqb))[:1], "k": list(range(nqb))[1:2],
                   "v": list(range(nqb))[2:]}[name]
            for nb in nbs:
                for i in range(8):
                    nc.sync.dma_start(xt_sb[i][:, ts(nb, 512)],
                                      xT[ts(i, 128), ts(nb, 512)])
        wo_sb = []
        for i in range(2):
            t = sbw.tile([128, D], MM, tag=f"wo{i}", name=f"wo{i}")
            nc.sync.dma_start(t[:], wo[ts(i, 128), :])
            wo_sb.append(t)

        # ---- persistent activations ----
        qT_sb = [sbqkv.tile([128, seq], MM, tag=f"qT{i}", name=f"qT{i}") for i in range(2)]
        kT_sb = [sbqkv.tile([128, seq], MM, tag=f"kT{i}", name=f"kT{i}") for i in range(2)]
        v_sb = [sbqkv.tile([128, NH * 65], MM, tag=f"v{i}", name=f"v{i}") for i in range(nst)]
        aT_sb = [sbqkv.tile([128, seq], MM, tag=f"aT{i}", name=f"aT{i}") for i in range(2)]

        # ---- projections: qT, kT ----
        def proj_qk(psp, name, bias_col, mt):
            dst = qT_sb if name == "q" else kT_sb
            for nb in range(nqb):
                ps = psp.tile([128, 512], F32, tag="pp", name="pp")
                for kt in range(8):
                    nc.tensor.matmul(
                        ps[:],
                        lhsT=w_sb[name][kt][:, ts(mt, 128)],
                        rhs=xt_sb[kt][:, ts(nb, 512)],
                        start=(kt == 0), stop=(kt == 7),
                    )
                nc.scalar.activation(
                    dst[mt][:, ts(nb, 512)], ps[:],
                    ACT.Identity, bias=bqk_sb[:, bias_col + mt:bias_col + mt + 1],
                )

        def proj_v(psp):
            for st in range(nst):
                ps = psp.tile([128, C], F32, tag="vp", name="vps")
                for kt in range(8):
                    nc.tensor.matmul(
                        ps[:],
                        lhsT=xt_sb[kt][:, ts(st, 128)],
                        rhs=w_sb["v"][kt][:],
                        start=(kt == 0), stop=(kt == 7),
                    )
                v3 = v_sb[st][:].rearrange("p (h e) -> p h e", e=65)
                nc.vector.tensor_copy(
                    v3[:, :, 64:65],
                    ones4[:].rearrange("p (h e) -> p h e", e=1))
                nc.vector.tensor_add(
                    v3[:, :, 0:64],
                    ps[:].rearrange("p (h e) -> p h e", e=64),
                    bvb[:].rearrange("p (h e) -> p h e", e=64),
                )
        # ---- attention + output projection ----
        def attn_head(scp, pvp, qh, h):
            tidx, poff = h // 2, 64 * (h % 2)
            qt, ktt = qT_sb[tidx], kT_sb[tidx]
            pv = pvp.tile([65, qh_w], F32, tag="pv", name="pv")
            for kt in range(nst):
                sc = scp.tile([128, qh_w], F32, tag="sc", name="sc")
                for qb in range(qh_b):
                    nc.tensor.matmul(
                        sc[:, ts(qb, 512)],
                        lhsT=ktt[poff:poff + 64, ts(kt, 128)],
                        rhs=qt[poff:poff + 64,
                               ds(qh * qh_w + qb * 512, 512)],
                        start=True, stop=True,
                    )
                pt = sbpt.tile([128, qh_w], MM, tag="pt", name="pt")
                nc.scalar.activation(pt[:], sc[:], ACT.Exp)
                for qb in range(qh_b):
                    nc.tensor.matmul(
                        pv[:, ts(qb, 512)],
                        lhsT=v_sb[kt][:, ds(65 * h, 65)],
                        rhs=pt[:, ts(qb, 512)],
                        start=(kt == 0), stop=(kt == nst - 1),
                    )
            recip = sbnrm.tile([1, qh_w], F32, tag="recip", name="recip")
            nc.vector.reciprocal(recip[:], pv[64:65, :])
            rbc = sbnrm.tile([64, qh_w], F32, tag="rbc", name="rbc")
            nc.gpsimd.partition_broadcast(rbc[:], recip[:])
            nc.vector.tensor_mul(
                aT_sb[tidx][poff:poff + 64, ds(qh * qh_w, qh_w)],
                pv[0:64, :], rbc[:],
            )

        def out_proj(scp, qh):
            for mt in range(8):
                for nb in range(qh * qh_b, (qh + 1) * qh_b):
                    yp = scp.tile([128, 512], F32, tag="sc", name="yp")
                    for kt2 in range(2):
                        nc.tensor.matmul(
                            yp[:],
                            lhsT=wo_sb[kt2][:, ts(mt, 128)],
                            rhs=aT_sb[kt2][:, ts(nb, 512)],
                            start=(kt2 == 0), stop=(kt2 == 1),
                        )
                    yt = sby.tile([128, 512], F32, tag="yt", name="yt")
                    nc.vector.tensor_copy(yt[:], yp[:])
                    nc.sync.dma_start(yT[ts(mt, 128), ts(nb, 512)], yt[:])

        with tc.tile_pool(name="pproj", bufs=3, space="PSUM") as psp:
            proj_qk(psp, "q", 0, 0)
            proj_qk(psp, "q", 0, 1)
            proj_qk(psp, "k", 2, 0)
            proj_qk(psp, "k", 2, 1)
            proj_v(psp)
        with (
            tc.tile_pool(name="psc", bufs=2, space="PSUM") as scp,
            tc.tile_pool(name="ppv", bufs=2, space="PSUM") as pvp,
        ):
            for qh in range(nqh):
                for h in range(NH):
                    attn_head(scp, pvp, qh, h)
                out_proj(scp, qh)

    nc.compile()
    return nc


def make_in_maps(x, Wq, bq, Wk, bk, Wv, bv, Wo):
    """Shard full inputs into 8 per-core input maps."""
    scale = np.float32(1.0 / np.sqrt(DH))
    xT = [np.ascontiguousarray(x[b].T) for b in range(2)]
    in_maps = []
    for c in range(8):
        b, g = c // 4, c % 4
        sl = slice(C * g, C * (g + 1))
        bq_g = (bq[sl] * scale).reshape(2, 128).T
        bk_g = bk[sl].reshape(2, 128).T
        in_maps.append({
            "xT": xT[b],
            "wq": np.ascontiguousarray(Wq[:, sl]) * scale,
            "wk": np.ascontiguousarray(Wk[:, sl]),
            "wv": np.ascontiguousarray(Wv[:, sl]),
            "wo": np.ascontiguousarray(Wo[sl, :]),
            "bqk": np.ascontiguousarray(
                np.concatenate([bq_g, bk_g], axis=1)).astype(np.float32),
            "bv": bv[sl].reshape(1, C).astype(np.float32),
        })
    return in_maps


def kernel(x, Wq, bq, Wk, bk, Wv, bv, Wo, bo):
    if os.environ.get("JAX_PLATFORMS") and \
            "axon" not in os.environ["JAX_PLATFORMS"]:
        os.environ.pop("JAX_PLATFORMS")
    trace = bool(os.environ.get("KERNEL_TRACE"))
    if trace:
        _install_ntff_shim()
    from concourse import bass_utils

    x = np.asarray(x, dtype=np.float32)
    in_maps = make_in_maps(
        x, np.asarray(Wq), np.asarray(bq), np.asarray(Wk), np.asarray(bk),
        np.asarray(Wv), np.asarray(bv), np.asarray(Wo))

    if "nc" not in _CACHE:
        _CACHE["nc"] = build_nc()
    res = bass_utils.run_bass_kernel_spmd(
        _CACHE["nc"], in_maps, core_ids=list(range(8)), trace=trace)
    _CACHE["exec_time_ns"] = res.exec_time_ns

    bo = np.asarray(bo, dtype=np.float32)
    out = np.empty((2, S, D), dtype=np.float32)
    for b in range(2):
        acc = res.results[4 * b]["yT"].copy()
        for g in range(1, 4):
            acc += res.results[4 * b + g]["yT"]
        out[b] = acc.T + bo
    return out

